# revision 6
# baseline (speedup 1.0000x reference)
# Multi-head attention kernel for 8 TRN2 NeuronCores.
#
# Sharding: data-parallel over batch. B=16 batches -> 2 per core; weights
# replicated; no collectives. Each core runs the full attention stack on
# its 2 batches.
#
# Per-core layout strategy (avoids transposing the [N,N] attention matrix):
#   - q,k,v,d,W transposed on-chip via TensorE-identity matmuls
#   - qh^T, kh^T = W^T.T @ x^T   (heads on partition axis, Dh=64 rows each)
#   - vh natural = x^T.T @ W^T   (seq on partition axis)
#   - scores^T[m,n] = kh^T.T @ qh^T  (softmax axis m = partitions)
#   - softmax denominator via ones-matmul on TensorE (partition reduction),
#     output pre-broadcast across 64 partitions
#   - att@v: x^T[dh,n] accumulated over m-tiles, normalized on PSUM evacuation
#   - out[n,e] = x^T.T @ Wp^T lands in natural layout for contiguous stores
# Matmul inputs are bitcast to float32r (fp32 storage, full PE rate).

import os
import numpy as np

B, N, E, H = 16, 1024, 512, 8
DH = E // H
NCORES = 8
BL = B // NCORES  # batches per core
P = 128  # partitions
NT = N // P  # 8 seq tiles
ET = E // P  # 4 embed tiles
NC2 = N // 512  # 2 n-chunks of 512

_graph_cache = {}


def build_graph():
    import concourse.bacc as bacc
    import concourse.tile as tile
    import concourse.mybir as mybir
    from concourse.masks import make_identity
    from contextlib import ExitStack

    dt = mybir.dt
    f32 = dt.float32
    f32r = dt.float32r
    AF = mybir.ActivationFunctionType

    def r(ap):  # matmul-input view
        return ap.bitcast(f32r)

    nc = bacc.Bacc(
        "TRN2", target_bir_lowering=False, debug=False, num_devices=NCORES
    )

    q_d = nc.dram_tensor("q", [BL, N, E], f32, kind="ExternalInput").ap()
    k_d = nc.dram_tensor("k", [BL, N, E], f32, kind="ExternalInput").ap()
    v_d = nc.dram_tensor("v", [BL, N, E], f32, kind="ExternalInput").ap()
    d_d = nc.dram_tensor("d", [BL, N, N], f32, kind="ExternalInput").ap()
    w_d = {
        w: nc.dram_tensor(w, [E, E], f32, kind="ExternalInput").ap()
        for w in ("Wq", "Wk", "Wv", "Wp")
    }
    b_d = {
        b: nc.dram_tensor(b, [E], f32, kind="ExternalInput").ap()
        for b in ("bq", "bk", "bv", "bp")
    }
    out_d = nc.dram_tensor("out", [BL, N, E], f32, kind="ExternalOutput").ap()

    with tile.TileContext(nc) as tc, ExitStack() as ctx:
        const = ctx.enter_context(tc.tile_pool(name="const", bufs=1))
        wpool = ctx.enter_context(tc.tile_pool(name="wts", bufs=1))
        natp = ctx.enter_context(tc.tile_pool(name="nat", bufs=3))
        actp = ctx.enter_context(tc.tile_pool(name="acts", bufs=1))
        smp = ctx.enter_context(tc.tile_pool(name="softmax", bufs=3))
        outp = ctx.enter_context(tc.tile_pool(name="outs", bufs=3))
        psp = ctx.enter_context(tc.tile_pool(name="ps", bufs=2, space="PSUM"))

        # ---- constants ----
        ident = const.tile([P, P], f32)
        make_identity(nc, ident[:])
        ones_tmp = const.tile([P, 64], f32, name="ones_tmp")
        nc.gpsimd.memset(ones_tmp[:], 1.0)
        ones64 = const.tile([P, 64], f32)
        nc.vector.tensor_copy(r(ones64[:]), ones_tmp[:])

        # biases as [128,1] column tiles (for T-layout adds, per-partition)
        bcol = {}
        for name in ("bq", "bk"):
            cols = []
            for ot in range(ET):
                c = const.tile([P, 1], f32, tag=f"bcol_{name}_{ot}")
                src = b_d[name].rearrange("(a b) -> a b", b=1)
                nc.sync.dma_start(c[:], src[ot * P : (ot + 1) * P, :])
                cols.append(c)
            bcol[name] = cols
        # scale bq by 1/sqrt(DH) (score scale folded into q path)
        for ot in range(ET):
            nc.vector.tensor_scalar_mul(
                bcol["bq"][ot][:], bcol["bq"][ot][:], 1.0 / (DH**0.5)
            )
        # biases broadcast to [128, E] (for natural-layout adds along free)
        brow = {}
        for name in ("bv", "bp"):
            rowt = const.tile([1, E], f32, tag=f"brow_{name}")
            nc.sync.dma_start(rowt[:], b_d[name].rearrange("(a b) -> a b", a=1))
            bb = const.tile([P, E], f32, tag=f"bb_{name}")
            nc.gpsimd.partition_broadcast(bb[:], rowt[:])
            brow[name] = bb

        # ---- weights: load natural [eo, ei], transpose to wT [ei, eo] ----
        wT = {}
        for name in ("Wq", "Wk", "Wv", "Wp"):
            tiles = []
            for et in range(ET):
                tiles.append(wpool.tile([P, E], f32, tag=f"wT_{name}_{et}", name=f"wT_{name}_{et}"))
            for ot in range(ET):
                wnat = natp.tile([P, E], f32, tag="xnat")
                nc.sync.dma_start(wnat[:], w_d[name][ot * P : (ot + 1) * P, :])
                for et in range(ET):
                    pst = psp.tile([P, P], f32, tag="ps_ts")
                    nc.tensor.transpose(
                        pst[:], wnat[:, et * P : (et + 1) * P], ident[:]
                    )
                    nc.vector.tensor_copy(
                        r(tiles[et][:, ot * P : (ot + 1) * P]), pst[:]
                    )
            wT[name] = tiles

        def load_and_transpose(x_dram, b, tag, scale=None):
            """[N, E'] natural in HBM -> list of ET' transposed SBUF tiles
            [128, N] (tag-shared storage)."""
            ecols = x_dram.shape[2]
            ets = ecols // P
            tiles = []
            for et in range(ets):
                tiles.append(actp.tile([P, N], f32, tag=f"{tag}_{et}", name=f"t_{tag}_{et}"))
            for nt in range(NT):
                xnat = natp.tile(
                    [P, ecols], f32, tag="xnat" if ecols == E else "dnat"
                )
                nc.sync.dma_start(xnat[:], x_dram[b, nt * P : (nt + 1) * P, :])
                for et in range(ets):
                    pst = psp.tile([P, P], f32, tag="ps_ts")
                    nc.tensor.transpose(
                        pst[:], xnat[:, et * P : (et + 1) * P], ident[:]
                    )
                    if scale is not None:
                        nc.vector.tensor_scalar_mul(
                            r(tiles[et][:, nt * P : (nt + 1) * P]), pst[:], scale
                        )
                    else:
                        nc.vector.tensor_copy(
                            r(tiles[et][:, nt * P : (nt + 1) * P]), pst[:]
                        )
            return tiles

        for b in range(BL):
            # ---- stages A+B interleaved per tensor (v, q, k share slots) --
            # v first: vT dead after vh projection, slots reused by qT/kT.
            vT = load_and_transpose(v_d, b, "xT")
            vh = []
            for mt in range(NT):
                vh.append(actp.tile([P, E], f32, tag=f"vh_{mt}", name=f"vh_{mt}"))
            for mt in range(NT):
                ps = psp.tile([P, 512], f32, tag="ps_proj")
                for et in range(ET):
                    nc.tensor.matmul(
                        ps[:],
                        r(vT[et][:, mt * P : (mt + 1) * P]),
                        r(wT["Wv"][et][:, :]),
                        start=(et == 0),
                        stop=(et == ET - 1),
                    )
                nc.vector.tensor_add(r(vh[mt][:]), ps[:], brow["bv"][:])

            hT = {}
            for xname, x_dram, wname, bname in (
                ("q", q_d, "Wq", "bq"),
                ("k", k_d, "Wk", "bk"),
            ):
                xT = load_and_transpose(
                    x_dram, b, "xT", scale=1.0 / (DH**0.5) if xname == "q" else None
                )
                tiles = []
                for ot in range(ET):
                    tiles.append(actp.tile([P, N], f32, tag=f"hT_{xname}_{ot}", name=f"hT_{xname}_{ot}"))
                for ot in range(ET):
                    for nch in range(NC2):
                        ps = psp.tile([P, 512], f32, tag="ps_proj")
                        for et in range(ET):
                            nc.tensor.matmul(
                                ps[:],
                                r(wT[wname][et][:, ot * P : (ot + 1) * P]),
                                r(xT[et][:, nch * 512 : (nch + 1) * 512]),
                                start=(et == 0),
                                stop=(et == ET - 1),
                            )
                        nc.vector.tensor_scalar_add(
                            r(tiles[ot][:, nch * 512 : (nch + 1) * 512]),
                            ps[:],
                            bcol[bname][ot][:],
                        )
                hT[xname] = tiles

            # d: transposed, reused across all heads
            dT = load_and_transpose(d_d, b, "dT")

            # ---- stage C: attention per head ----
            x_T = []
            for et in range(ET):
                x_T.append(actp.tile([P, N], f32, tag=f"xT_{et}", name=f"x_T_{et}"))
            for h in range(H):
                ht, r0 = h // 2, (h % 2) * 64
                for ncc in range(NC2):
                    nsl = slice(ncc * 512, (ncc + 1) * 512)
                    ps_sum = psp.tile([P, 512], f32, tag="ps_sum")
                    ps_x = psp.tile([P, 512], f32, tag="ps_x")
                    for mt in range(NT):
                        ps_s = psp.tile([P, 512], f32, tag="ps_ts")
                        nc.tensor.matmul(
                            ps_s[:],
                            r(hT["k"][ht][r0 : r0 + 64, mt * P : (mt + 1) * P]),
                            r(hT["q"][ht][r0 : r0 + 64, nsl]),
                            start=True,
                            stop=True,
                        )
                        sprime = smp.tile([P, 512], f32, tag="sprime")
                        nc.vector.tensor_add(sprime[:], ps_s[:], dT[mt][:, nsl])
                        expt = smp.tile([P, 512], f32, tag="expt")
                        nc.scalar.activation(r(expt[:]), sprime[:], AF.Exp)
                        nc.tensor.matmul(
                            ps_sum[0:64, :],
                            r(ones64[:]),
                            r(expt[:]),
                            start=(mt == 0),
                            stop=(mt == NT - 1),
                            skip_group_check=True,
                        )
                        pt = smp.tile([P, 512], f32, tag="pt")
                        nc.vector.tensor_mul(r(pt[:]), expt[:], dT[mt][:, nsl])
                        nc.tensor.matmul(
                            ps_x[0:64, :],
                            r(vh[mt][:, h * 64 : (h + 1) * 64]),
                            r(pt[:]),
                            start=(mt == 0),
                            stop=(mt == NT - 1),
                            skip_group_check=True,
                        )
                    recip = smp.tile([P, 512], f32, tag="recip")
                    nc.vector.reciprocal(recip[0:64, :], ps_sum[0:64, :])
                    if r0 == 0:
                        nc.vector.tensor_mul(
                            r(x_T[ht][0:64, nsl]), ps_x[0:64, :], recip[0:64, :]
                        )
                    else:
                        xtmp = smp.tile([64, 512], f32, tag="xtmp")
                        nc.vector.tensor_mul(
                            r(xtmp[:]), ps_x[0:64, :], recip[0:64, :]
                        )
                        nc.sync.dma_start(
                            r(x_T[ht][64:128, nsl]), r(xtmp[:])
                        )

            # ---- stage D: output projection ----
            for nt in range(NT):
                ps = psp.tile([P, 512], f32, tag="ps_proj")
                for et in range(ET):
                    nc.tensor.matmul(
                        ps[:],
                        r(x_T[et][:, nt * P : (nt + 1) * P]),
                        r(wT["Wp"][et][:, :]),
                        start=(et == 0),
                        stop=(et == ET - 1),
                    )
                ot_sb = outp.tile([P, E], f32, tag="ot_sb")
                nc.vector.tensor_add(ot_sb[:], ps[:], brow["bp"][:])
                nc.sync.dma_start(out_d[b, nt * P : (nt + 1) * P, :], ot_sb[:])

    nc.compile()
    return nc


def _get_graph():
    if "nc" not in _graph_cache:
        _graph_cache["nc"] = build_graph()
    return _graph_cache["nc"]


def make_in_maps(full):
    in_maps = []
    for c in range(NCORES):
        bsl = slice(c * BL, (c + 1) * BL)
        m = {
            "q": full["q"][bsl],
            "k": full["k"][bsl],
            "v": full["v"][bsl],
            "d": full["d"][bsl],
        }
        for w in ("Wq", "Wk", "Wv", "Wp", "bq", "bk", "bv", "bp"):
            m[w] = full[w]
        in_maps.append(m)
    return in_maps


def kernel(**inputs):
    from concourse.bass_utils import run_bass_kernel_spmd

    nc = _get_graph()
    full = {
        k: np.ascontiguousarray(np.asarray(v, np.float32))
        for k, v in inputs.items()
    }
    res = run_bass_kernel_spmd(
        nc,
        make_in_maps(full),
        core_ids=list(range(NCORES)),
        trace=bool(os.environ.get("ATTN_TRACE")),
    )
    if res.exec_time_ns is not None:
        _graph_cache["exec_time_ns"] = res.exec_time_ns
        _graph_cache["profile_json"] = res.profile_json
        _graph_cache["trace"] = res.instructions_and_trace
    out = np.concatenate([res.results[c]["out"] for c in range(NCORES)], axis=0)
    return out


# revision 10
# speedup vs baseline: 1.6022x; 1.6022x over previous
# Multi-head attention kernel for 8 TRN2 NeuronCores.
#
# Sharding: data-parallel over batch. B=16 batches -> 2 per core; weights
# replicated; no collectives. Each core runs the full attention stack on
# its 2 batches.
#
# v2 design (bf16 compute, fp32 accumulate):
#   - inputs cast f32->bf16 during the HBM->SBUF DMA itself (SWDGE cast)
#   - q,k,v,d,W transposed by batched HWDGE transpose-DMAs (no PE work)
#   - qh^T, kh^T = W^T.T @ q^T; vh natural = v^T.T @ Wv^T   (bf16 matmuls)
#   - scores^T[m,n] = kh^T.T @ qh^T per head; head PAIRS packed into the
#     PE array (rows 0-63 / 64-127), their softmax stats and att@v
#     col-packed via tile_position (0,0)/(0,64) into shared PSUM banks
#   - softmax uses exp(s+d) = exp(s)*exp(d): g=exp(d^T), f=d*exp(d) are
#     precomputed per batch, so both DVE passes run at 2x bf16 mode:
#       TT1 = e*g feeds the ones-matmul denominator, TT2 = e*f feeds att@v
#   - 1/sums via ScalarE ln then exp(-x) (DVE iterative divide is ~8x slower)
#   - x^T normalized on PSUM evacuation; out = x^T.T @ Wp^T lands natural
#   - biases are all-zero per the problem spec; accepted but not added
import os
import numpy as np

B, N, E, H = 16, 1024, 512, 8
DH = E // H
NCORES = 8
BL = B // NCORES  # batches per core
P = 128
NT = N // P  # 8 seq tiles
ET = E // P  # 4 embed tiles
NC2 = N // 512  # 2 n-chunks of 512
HP = H // 2  # 4 head pairs

_graph_cache = {}


def build_graph():
    import concourse.bacc as bacc
    import concourse.tile as tile
    import concourse.mybir as mybir
    from contextlib import ExitStack

    dt = mybir.dt
    f32 = dt.float32
    bf16 = dt.bfloat16
    AF = mybir.ActivationFunctionType

    nc = bacc.Bacc(
        "TRN2", target_bir_lowering=False, debug=False, num_devices=NCORES
    )

    q_d = nc.dram_tensor("q", [BL, N, E], f32, kind="ExternalInput").ap()
    k_d = nc.dram_tensor("k", [BL, N, E], f32, kind="ExternalInput").ap()
    v_d = nc.dram_tensor("v", [BL, N, E], f32, kind="ExternalInput").ap()
    d_d = nc.dram_tensor("d", [BL, N, N], f32, kind="ExternalInput").ap()
    w_d = {
        w: nc.dram_tensor(w, [E, E], f32, kind="ExternalInput").ap()
        for w in ("Wq", "Wk", "Wv", "Wp")
    }
    for bname in ("bq", "bk", "bv", "bp"):
        # all-zero per the problem spec; declared so the harness can bind them
        nc.dram_tensor(bname, [E], f32, kind="ExternalInput")
    out_d = nc.dram_tensor("out", [BL, N, E], f32, kind="ExternalOutput").ap()

    with tile.TileContext(nc) as tc, ExitStack() as ctx:
        wpool = ctx.enter_context(tc.tile_pool(name="wts", bufs=1))
        dram = ctx.enter_context(tc.tile_pool(name="dram", bufs=1, space="DRAM"))
        actp = ctx.enter_context(tc.tile_pool(name="acts", bufs=1))
        smp = ctx.enter_context(tc.tile_pool(name="softmax", bufs=3))
        outp = ctx.enter_context(tc.tile_pool(name="outs", bufs=3))
        psp = ctx.enter_context(tc.tile_pool(name="ps", bufs=2, space="PSUM"))

        ones64 = wpool.tile([P, 64], bf16)
        nc.gpsimd.memset(ones64[:], 1.0)

        # ---- weights: DRAM cast f32->bf16, then transpose-load wT [ei, eo] --
        wT = {}
        for name in ("Wq", "Wk", "Wv", "Wp"):
            wbf = dram.tile([E, E], bf16, tag=f"wbf_{name}", name=f"wbf_{name}")
            nc.gpsimd.dma_start(wbf[:], w_d[name][:])
            tiles = []
            for et in range(ET):
                t = wpool.tile(
                    [P, E], bf16, tag=f"wT_{name}_{et}", name=f"wT_{name}_{et}"
                )
                nc.sync.dma_start(
                    t[:], wbf[:, et * P : (et + 1) * P], transpose=True
                )
                tiles.append(t)
            wT[name] = tiles

        def cast_load_transpose(x_dram, b, tag, ets):
            """DRAM [N, ets*128] f32 -> DRAM bf16 bounce -> ets transposed
            bf16 SBUF tiles [128, N] via DRAM-side transpose-loads."""
            ecols = ets * P
            xbf = dram.tile([N, ecols], bf16, tag=f"bf_{tag}", name=f"bf_{tag}")
            nc.gpsimd.dma_start(xbf[:], x_dram[b])
            tiles = []
            for et in range(ets):
                t = actp.tile(
                    [P, N], bf16, tag=f"{tag}_{et}", name=f"t_{tag}_{et}"
                )
                nc.sync.dma_start(
                    t[:], xbf[:, et * P : (et + 1) * P], transpose=True
                )
                tiles.append(t)
            return tiles

        for b in range(BL):
            # ---- load + transpose activations ----
            vT = cast_load_transpose(v_d, b, "vT", ET)
            qT = cast_load_transpose(q_d, b, "qT", ET)
            kT = cast_load_transpose(k_d, b, "kT", ET)
            dT = cast_load_transpose(d_d, b, "dT", NT)

            # g = exp(d^T), f = d^T * exp(d^T)
            gT, fT = [], []
            for mt in range(NT):
                g = actp.tile([P, N], bf16, tag=f"gT_{mt}", name=f"gT_{mt}")
                nc.scalar.activation(g[:], dT[mt][:], AF.Exp)
                f = actp.tile([P, N], bf16, tag=f"fT_{mt}", name=f"fT_{mt}")
                nc.vector.tensor_mul(f[:], g[:], dT[mt][:])
                gT.append(g)
                fT.append(f)

            # ---- projections ----
            hT = {}
            for xname, xT, wname in (("q", qT, "Wq"), ("k", kT, "Wk")):
                tiles = []
                for ot in range(ET):
                    tiles.append(
                        actp.tile(
                            [P, N],
                            bf16,
                            tag=f"hT_{xname}_{ot}",
                            name=f"hT_{xname}_{ot}",
                        )
                    )
                for ot in range(ET):
                    for nch in range(NC2):
                        ps = psp.tile([P, 512], f32, tag="ps_s", bufs=4)
                        for et in range(ET):
                            nc.tensor.matmul(
                                ps[:],
                                wT[wname][et][:, ot * P : (ot + 1) * P],
                                xT[et][:, nch * 512 : (nch + 1) * 512],
                                start=(et == 0),
                                stop=(et == ET - 1),
                            )
                        dst = tiles[ot][:, nch * 512 : (nch + 1) * 512]
                        if xname == "q":
                            nc.vector.tensor_scalar_mul(
                                dst, ps[:], 1.0 / (DH**0.5)
                            )
                        else:
                            nc.vector.tensor_copy(dst, ps[:])
                hT[xname] = tiles

            vh = []
            for mt in range(NT):
                vh.append(
                    actp.tile([P, E], bf16, tag=f"vh_{mt}", name=f"vh_{mt}")
                )
            for mt in range(NT):
                ps = psp.tile([P, 512], f32, tag="ps_s", bufs=4)
                for et in range(ET):
                    nc.tensor.matmul(
                        ps[:],
                        vT[et][:, mt * P : (mt + 1) * P],
                        wT["Wv"][et][:, :],
                        start=(et == 0),
                        stop=(et == ET - 1),
                    )
                nc.vector.tensor_copy(vh[mt][:], ps[:])

            # ---- attention, head pairs packed ----
            x_T = []
            for hp in range(HP):
                x_T.append(
                    actp.tile([P, N], bf16, tag=f"xT_{hp}", name=f"x_T_{hp}")
                )
            for hp in range(HP):
                h0, h1 = 2 * hp, 2 * hp + 1
                for ncc in range(NC2):
                    nsl = slice(ncc * 512, (ncc + 1) * 512)
                    ps_sum = psp.tile([P, 512], f32, tag="ps_sum")
                    ps_x = psp.tile([P, 512], f32, tag="ps_x")
                    for mt in range(NT):
                        msl = slice(mt * P, (mt + 1) * P)
                        ps_s0 = psp.tile([P, 512], f32, tag="ps_s", bufs=4)
                        ps_s1 = psp.tile([P, 512], f32, tag="ps_s", bufs=4)
                        nc.tensor.matmul(
                            ps_s0[:],
                            hT["k"][hp][0:64, msl],
                            hT["q"][hp][0:64, nsl],
                            start=True,
                            stop=True,
                        )
                        nc.tensor.matmul(
                            ps_s1[:],
                            hT["k"][hp][64:128, msl],
                            hT["q"][hp][64:128, nsl],
                            start=True,
                            stop=True,
                        )
                        e0 = smp.tile([P, 512], bf16, tag="e0")
                        e1 = smp.tile([P, 512], bf16, tag="e1")
                        nc.scalar.activation(e0[:], ps_s0[:], AF.Exp)
                        nc.scalar.activation(e1[:], ps_s1[:], AF.Exp)
                        t10 = smp.tile([P, 512], bf16, tag="t10")
                        t11 = smp.tile([P, 512], bf16, tag="t11")
                        nc.vector.tensor_mul(t10[:], e0[:], gT[mt][:, nsl])
                        nc.vector.tensor_mul(t11[:], e1[:], gT[mt][:, nsl])
                        nc.tensor.matmul(
                            ps_sum[0:64, :],
                            ones64[:],
                            t10[:],
                            start=(mt == 0),
                            stop=(mt == NT - 1),
                            skip_group_check=True,
                        )
                        nc.tensor.matmul(
                            ps_sum[64:128, :],
                            ones64[:],
                            t11[:],
                            start=(mt == 0),
                            stop=(mt == NT - 1),
                            skip_group_check=True,
                            tile_position=(0, 64),
                        )
                        t20 = smp.tile([P, 512], bf16, tag="t20")
                        t21 = smp.tile([P, 512], bf16, tag="t21")
                        nc.vector.tensor_mul(t20[:], e0[:], fT[mt][:, nsl])
                        nc.vector.tensor_mul(t21[:], e1[:], fT[mt][:, nsl])
                        nc.tensor.matmul(
                            ps_x[0:64, :],
                            vh[mt][:, h0 * 64 : h0 * 64 + 64],
                            t20[:],
                            start=(mt == 0),
                            stop=(mt == NT - 1),
                            skip_group_check=True,
                        )
                        nc.tensor.matmul(
                            ps_x[64:128, :],
                            vh[mt][:, h1 * 64 : h1 * 64 + 64],
                            t21[:],
                            start=(mt == 0),
                            stop=(mt == NT - 1),
                            skip_group_check=True,
                            tile_position=(0, 64),
                        )
                    lnt = smp.tile([P, 512], f32, tag="lnt")
                    nc.scalar.activation(lnt[:], ps_sum[:], AF.Ln)
                    rec = smp.tile([P, 512], f32, tag="rec")
                    nc.scalar.activation(rec[:], lnt[:], AF.Exp, scale=-1.0)
                    nc.vector.tensor_mul(x_T[hp][:, nsl], ps_x[:], rec[:])

            # ---- output projection ----
            for nt in range(NT):
                ps = psp.tile([P, 512], f32, tag="ps_s", bufs=4)
                for hp in range(HP):
                    nc.tensor.matmul(
                        ps[:],
                        x_T[hp][:, nt * P : (nt + 1) * P],
                        wT["Wp"][hp][:, :],
                        start=(hp == 0),
                        stop=(hp == HP - 1),
                    )
                ot_sb = outp.tile([P, E], f32, tag="ot_sb")
                nc.vector.tensor_copy(ot_sb[:], ps[:])
                nc.sync.dma_start(out_d[b, nt * P : (nt + 1) * P, :], ot_sb[:])

    nc.compile()
    return nc


def _get_graph():
    if "nc" not in _graph_cache:
        _graph_cache["nc"] = build_graph()
    return _graph_cache["nc"]


def make_in_maps(full):
    in_maps = []
    for c in range(NCORES):
        bsl = slice(c * BL, (c + 1) * BL)
        m = {
            "q": full["q"][bsl],
            "k": full["k"][bsl],
            "v": full["v"][bsl],
            "d": full["d"][bsl],
        }
        for w in ("Wq", "Wk", "Wv", "Wp", "bq", "bk", "bv", "bp"):
            m[w] = full[w]
        in_maps.append(m)
    return in_maps


def kernel(**inputs):
    from concourse.bass_utils import run_bass_kernel_spmd

    nc = _get_graph()
    full = {
        k: np.ascontiguousarray(np.asarray(v, np.float32))
        for k, v in inputs.items()
    }
    res = run_bass_kernel_spmd(
        nc,
        make_in_maps(full),
        core_ids=list(range(NCORES)),
        trace=bool(os.environ.get("ATTN_TRACE")),
    )
    if res.exec_time_ns is not None:
        _graph_cache["exec_time_ns"] = res.exec_time_ns
        _graph_cache["profile_json"] = res.profile_json
        _graph_cache["trace"] = res.instructions_and_trace
    out = np.concatenate([res.results[c]["out"] for c in range(NCORES)], axis=0)
    return out


# revision 12
# speedup vs baseline: 1.7524x; 1.0938x over previous
# Multi-head attention kernel for 8 TRN2 NeuronCores.
#
# Sharding: data-parallel over batch. B=16 batches -> 2 per core; weights
# replicated; no collectives. Each core runs the full attention stack on
# its 2 batches.
#
# v2 design (bf16 compute, fp32 accumulate):
#   - inputs cast f32->bf16 during the HBM->SBUF DMA itself (SWDGE cast)
#   - q,k,v,d,W transposed by batched HWDGE transpose-DMAs (no PE work)
#   - qh^T, kh^T = W^T.T @ q^T; vh natural = v^T.T @ Wv^T   (bf16 matmuls)
#   - scores^T[m,n] = kh^T.T @ qh^T per head; head PAIRS packed into the
#     PE array (rows 0-63 / 64-127), their softmax stats and att@v
#     col-packed via tile_position (0,0)/(0,64) into shared PSUM banks
#   - softmax uses exp(s+d) = exp(s)*exp(d): g=exp(d^T), f=d*exp(d) are
#     precomputed per batch, so both DVE passes run at 2x bf16 mode:
#       TT1 = e*g feeds the ones-matmul denominator, TT2 = e*f feeds att@v
#   - 1/sums via ScalarE ln then exp(-x) (DVE iterative divide is ~8x slower)
#   - x^T normalized on PSUM evacuation; out = x^T.T @ Wp^T lands natural
#   - biases are all-zero per the problem spec; accepted but not added
import os
import numpy as np

B, N, E, H = 16, 1024, 512, 8
DH = E // H
NCORES = 8
BL = B // NCORES  # batches per core
P = 128
NT = N // P  # 8 seq tiles
ET = E // P  # 4 embed tiles
NC2 = N // 512  # 2 n-chunks of 512
HP = H // 2  # 4 head pairs

_graph_cache = {}


def build_graph():
    import concourse.bacc as bacc
    import concourse.tile as tile
    import concourse.mybir as mybir
    from contextlib import ExitStack

    dt = mybir.dt
    f32 = dt.float32
    bf16 = dt.bfloat16
    AF = mybir.ActivationFunctionType

    nc = bacc.Bacc(
        "TRN2", target_bir_lowering=False, debug=False, num_devices=NCORES
    )

    q_d = nc.dram_tensor("q", [BL, N, E], f32, kind="ExternalInput").ap()
    k_d = nc.dram_tensor("k", [BL, N, E], f32, kind="ExternalInput").ap()
    v_d = nc.dram_tensor("v", [BL, N, E], f32, kind="ExternalInput").ap()
    d_d = nc.dram_tensor("d", [BL, N, N], f32, kind="ExternalInput").ap()
    w_d = {
        w: nc.dram_tensor(w, [E, E], f32, kind="ExternalInput").ap()
        for w in ("Wq", "Wk", "Wv", "Wp")
    }
    for bname in ("bq", "bk", "bv", "bp"):
        # all-zero per the problem spec; declared so the harness can bind them
        nc.dram_tensor(bname, [E], f32, kind="ExternalInput")
    out_d = nc.dram_tensor("out", [BL, N, E], f32, kind="ExternalOutput").ap()

    with tile.TileContext(nc) as tc, ExitStack() as ctx:
        wpool = ctx.enter_context(tc.tile_pool(name="wts", bufs=1))
        dram = ctx.enter_context(tc.tile_pool(name="dram", bufs=1, space="DRAM"))
        actp = ctx.enter_context(tc.tile_pool(name="acts", bufs=1))
        smp = ctx.enter_context(tc.tile_pool(name="softmax", bufs=3))
        outp = ctx.enter_context(tc.tile_pool(name="outs", bufs=3))
        psp = ctx.enter_context(tc.tile_pool(name="ps", bufs=2, space="PSUM"))

        ones64 = wpool.tile([P, 64], bf16)
        nc.gpsimd.memset(ones64[:], 1.0)

        # ---- weights: DRAM cast f32->bf16, then transpose-load wT [ei, eo] --
        wT = {}
        for name in ("Wq", "Wk", "Wv", "Wp"):
            wbf = dram.tile([E, E], bf16, tag=f"wbf_{name}", name=f"wbf_{name}")
            nc.gpsimd.dma_start(wbf[:], w_d[name][:])
            tiles = []
            for et in range(ET):
                t = wpool.tile(
                    [P, E], bf16, tag=f"wT_{name}_{et}", name=f"wT_{name}_{et}"
                )
                nc.sync.dma_start(
                    t[:], wbf[:, et * P : (et + 1) * P], transpose=True
                )
                tiles.append(t)
            wT[name] = tiles

        def cast_load_transpose(x_dram, b, tag, ets):
            """DRAM [N, ets*128] f32 -> DRAM bf16 bounce -> ets transposed
            bf16 SBUF tiles [128, N] via DRAM-side transpose-loads."""
            ecols = ets * P
            xbf = dram.tile([N, ecols], bf16, tag=f"bf_{tag}", name=f"bf_{tag}")
            nc.gpsimd.dma_start(xbf[:], x_dram[b])
            tiles = []
            for et in range(ets):
                t = actp.tile(
                    [P, N], bf16, tag=f"{tag}_{et}", name=f"t_{tag}_{et}"
                )
                nc.sync.dma_start(
                    t[:], xbf[:, et * P : (et + 1) * P], transpose=True
                )
                tiles.append(t)
            return tiles

        for b in range(BL):
            # ---- load + transpose activations ----
            vT = cast_load_transpose(v_d, b, "vT", ET)
            qT = cast_load_transpose(q_d, b, "qT", ET)
            kT = cast_load_transpose(k_d, b, "kT", ET)
            dT = cast_load_transpose(d_d, b, "dT", NT)

            # g = exp(d^T), f = d^T * exp(d^T)
            gT, fT = [], []
            for mt in range(NT):
                g = actp.tile([P, N], bf16, tag=f"gT_{mt}", name=f"gT_{mt}")
                nc.scalar.activation(g[:], dT[mt][:], AF.Exp)
                f = actp.tile([P, N], bf16, tag=f"fT_{mt}", name=f"fT_{mt}")
                nc.vector.tensor_mul(f[:], g[:], dT[mt][:])
                gT.append(g)
                fT.append(f)

            # ---- projections (both 512-chunks share one 2-bank psum) ----
            hT = {}
            for xname, xT, wname in (("q", qT, "Wq"), ("k", kT, "Wk")):
                tiles = []
                for ot in range(ET):
                    tiles.append(
                        actp.tile(
                            [P, N],
                            bf16,
                            tag=f"hT_{xname}_{ot}",
                            name=f"hT_{xname}_{ot}",
                        )
                    )
                for ot in range(ET):
                    ps = psp.tile([P, 1024], f32, tag="ps_pair")
                    for nch in range(NC2):
                        for et in range(ET):
                            nc.tensor.matmul(
                                ps[:, nch * 512 : (nch + 1) * 512],
                                wT[wname][et][:, ot * P : (ot + 1) * P],
                                xT[et][:, nch * 512 : (nch + 1) * 512],
                                start=(et == 0),
                                stop=(et == ET - 1),
                            )
                    if xname == "q":
                        nc.vector.tensor_scalar_mul(
                            tiles[ot][:], ps[:], 1.0 / (DH**0.5)
                        )
                    else:
                        nc.vector.tensor_copy(tiles[ot][:], ps[:])
                hT[xname] = tiles

            # vh as one [128, NT*E] tensor; mt pairs share one 2-bank psum
            vh_all = actp.tile([P, NT * E], bf16, tag="vh_all", name="vh_all")
            for mtp in range(NT // 2):
                ps = psp.tile([P, 1024], f32, tag="ps_pair")
                for j in range(2):
                    mt = 2 * mtp + j
                    for et in range(ET):
                        nc.tensor.matmul(
                            ps[:, j * 512 : (j + 1) * 512],
                            vT[et][:, mt * P : (mt + 1) * P],
                            wT["Wv"][et][:, :],
                            start=(et == 0),
                            stop=(et == ET - 1),
                        )
                nc.vector.tensor_copy(
                    vh_all[:, mtp * 1024 : (mtp + 1) * 1024], ps[:]
                )

            # ---- attention: head pairs in one [128,1024] pipeline ----
            x_all = actp.tile([P, HP * N], bf16, tag="x_all", name="x_all")
            sums_all = actp.tile(
                [P, H * 512], f32, tag="sums_all", name="sums_all"
            )
            xu_all = actp.tile([P, H * 512], bf16, tag="xu_all", name="xu_all")
            for hp in range(HP):
                h0, h1 = 2 * hp, 2 * hp + 1
                for ncc in range(NC2):
                    nsl = slice(ncc * 512, (ncc + 1) * 512)
                    slot = hp * 2 + ncc
                    ps_sum = psp.tile([P, 512], f32, tag="ps_sum")
                    ps_x = psp.tile([P, 512], f32, tag="ps_x")
                    for mt in range(NT):
                        msl = slice(mt * P, (mt + 1) * P)
                        ps_pair = psp.tile([P, 1024], f32, tag="ps_pair")
                        nc.tensor.matmul(
                            ps_pair[:, 0:512],
                            hT["k"][hp][0:64, msl],
                            hT["q"][hp][0:64, nsl],
                            start=True,
                            stop=True,
                        )
                        nc.tensor.matmul(
                            ps_pair[:, 512:1024],
                            hT["k"][hp][64:128, msl],
                            hT["q"][hp][64:128, nsl],
                            start=True,
                            stop=True,
                        )
                        e01 = smp.tile([P, 1024], bf16, tag="e01")
                        nc.scalar.activation(e01[:], ps_pair[:], AF.Exp)
                        gb = (
                            gT[mt][:, nsl]
                            .rearrange("p (o f) -> p o f", o=1)
                            .broadcast_to((P, 2, 512))
                        )
                        fb = (
                            fT[mt][:, nsl]
                            .rearrange("p (o f) -> p o f", o=1)
                            .broadcast_to((P, 2, 512))
                        )
                        e2 = e01[:].rearrange("p (o f) -> p o f", o=2)
                        t1 = smp.tile([P, 1024], bf16, tag="t1")
                        nc.vector.tensor_mul(
                            t1[:].rearrange("p (o f) -> p o f", o=2), e2, gb
                        )
                        nc.tensor.matmul(
                            ps_sum[0:64, :],
                            ones64[:],
                            t1[:, 0:512],
                            start=(mt == 0),
                            stop=(mt == NT - 1),
                            skip_group_check=True,
                        )
                        nc.tensor.matmul(
                            ps_sum[64:128, :],
                            ones64[:],
                            t1[:, 512:1024],
                            start=(mt == 0),
                            stop=(mt == NT - 1),
                            skip_group_check=True,
                            tile_position=(0, 64),
                        )
                        t2 = smp.tile([P, 1024], bf16, tag="t2")
                        nc.vector.tensor_mul(
                            t2[:].rearrange("p (o f) -> p o f", o=2), e2, fb
                        )
                        nc.tensor.matmul(
                            ps_x[0:64, :],
                            vh_all[:, mt * 512 + h0 * 64 : mt * 512 + h0 * 64 + 64],
                            t2[:, 0:512],
                            start=(mt == 0),
                            stop=(mt == NT - 1),
                            skip_group_check=True,
                        )
                        nc.tensor.matmul(
                            ps_x[64:128, :],
                            vh_all[:, mt * 512 + h1 * 64 : mt * 512 + h1 * 64 + 64],
                            t2[:, 512:1024],
                            start=(mt == 0),
                            stop=(mt == NT - 1),
                            skip_group_check=True,
                            tile_position=(0, 64),
                        )
                    nc.vector.tensor_copy(
                        sums_all[:, slot * 512 : (slot + 1) * 512], ps_sum[:]
                    )
                    nc.vector.tensor_copy(
                        xu_all[:, slot * 512 : (slot + 1) * 512], ps_x[:]
                    )
            # batched reciprocal: rec = exp(-ln(sums)); then one normalize op
            lnt = smp.tile([P, H * 512], f32, tag="lnt", bufs=1)
            nc.scalar.activation(lnt[:], sums_all[:], AF.Ln)
            nc.scalar.activation(lnt[:], lnt[:], AF.Exp, scale=-1.0)
            nc.vector.tensor_mul(x_all[:], xu_all[:], lnt[:])

            # ---- output projection (nt pairs share one 2-bank psum) ----
            for ntp in range(NT // 2):
                ps = psp.tile([P, 1024], f32, tag="ps_pair")
                for j in range(2):
                    nt = 2 * ntp + j
                    for hp in range(HP):
                        # x_all[:, hp*N + nt*P : ...] rows = heads 2hp,2hp+1
                        nc.tensor.matmul(
                            ps[:, j * 512 : (j + 1) * 512],
                            x_all[:, hp * N + nt * P : hp * N + (nt + 1) * P],
                            wT["Wp"][hp][:, :],
                            start=(hp == 0),
                            stop=(hp == HP - 1),
                        )
                ot_sb = outp.tile([P, 1024], f32, tag="ot_sb", bufs=2)
                nc.vector.tensor_copy(ot_sb[:], ps[:])
                nc.sync.dma_start(
                    out_d[
                        b, ntp * 2 * P : (ntp + 1) * 2 * P, :
                    ].rearrange("(c p) e -> p c e", p=P),
                    ot_sb[:].rearrange("p (c e) -> p c e", c=2),
                )

    nc.compile()
    return nc


def _get_graph():
    if "nc" not in _graph_cache:
        _graph_cache["nc"] = build_graph()
    return _graph_cache["nc"]


def make_in_maps(full):
    in_maps = []
    for c in range(NCORES):
        bsl = slice(c * BL, (c + 1) * BL)
        m = {
            "q": full["q"][bsl],
            "k": full["k"][bsl],
            "v": full["v"][bsl],
            "d": full["d"][bsl],
        }
        for w in ("Wq", "Wk", "Wv", "Wp", "bq", "bk", "bv", "bp"):
            m[w] = full[w]
        in_maps.append(m)
    return in_maps


def kernel(**inputs):
    from concourse.bass_utils import run_bass_kernel_spmd

    nc = _get_graph()
    full = {
        k: np.ascontiguousarray(np.asarray(v, np.float32))
        for k, v in inputs.items()
    }
    res = run_bass_kernel_spmd(
        nc,
        make_in_maps(full),
        core_ids=list(range(NCORES)),
        trace=bool(os.environ.get("ATTN_TRACE")),
    )
    if res.exec_time_ns is not None:
        _graph_cache["exec_time_ns"] = res.exec_time_ns
        _graph_cache["profile_json"] = res.profile_json
        _graph_cache["trace"] = res.instructions_and_trace
    out = np.concatenate([res.results[c]["out"] for c in range(NCORES)], axis=0)
    return out


# revision 13
# speedup vs baseline: 1.8556x; 1.0588x over previous
# Multi-head attention kernel for 8 TRN2 NeuronCores.
#
# Sharding: data-parallel over batch. B=16 batches -> 2 per core; weights
# replicated; no collectives. Each core runs the full attention stack on
# its 2 batches.
#
# v2 design (bf16 compute, fp32 accumulate):
#   - inputs cast f32->bf16 during the HBM->SBUF DMA itself (SWDGE cast)
#   - q,k,v,d,W transposed by batched HWDGE transpose-DMAs (no PE work)
#   - qh^T, kh^T = W^T.T @ q^T; vh natural = v^T.T @ Wv^T   (bf16 matmuls)
#   - scores^T[m,n] = kh^T.T @ qh^T per head; head PAIRS packed into the
#     PE array (rows 0-63 / 64-127), their softmax stats and att@v
#     col-packed via tile_position (0,0)/(0,64) into shared PSUM banks
#   - softmax uses exp(s+d) = exp(s)*exp(d): g=exp(d^T), f=d*exp(d) are
#     precomputed per batch, so both DVE passes run at 2x bf16 mode:
#       TT1 = e*g feeds the ones-matmul denominator, TT2 = e*f feeds att@v
#   - 1/sums via ScalarE ln then exp(-x) (DVE iterative divide is ~8x slower)
#   - x^T normalized on PSUM evacuation; out = x^T.T @ Wp^T lands natural
#   - biases are all-zero per the problem spec; accepted but not added
import os
import numpy as np

B, N, E, H = 16, 1024, 512, 8
DH = E // H
NCORES = 8
BL = B // NCORES  # batches per core
P = 128
NT = N // P  # 8 seq tiles
ET = E // P  # 4 embed tiles
NC2 = N // 512  # 2 n-chunks of 512
HP = H // 2  # 4 head pairs

_graph_cache = {}


def build_graph():
    import concourse.bacc as bacc
    import concourse.tile as tile
    import concourse.mybir as mybir
    from contextlib import ExitStack

    dt = mybir.dt
    f32 = dt.float32
    bf16 = dt.bfloat16
    AF = mybir.ActivationFunctionType

    nc = bacc.Bacc(
        "TRN2", target_bir_lowering=False, debug=False, num_devices=NCORES
    )

    q_d = nc.dram_tensor("q", [BL, N, E], f32, kind="ExternalInput").ap()
    k_d = nc.dram_tensor("k", [BL, N, E], f32, kind="ExternalInput").ap()
    v_d = nc.dram_tensor("v", [BL, N, E], f32, kind="ExternalInput").ap()
    d_d = nc.dram_tensor("d", [BL, N, N], f32, kind="ExternalInput").ap()
    w_d = {
        w: nc.dram_tensor(w, [E, E], f32, kind="ExternalInput").ap()
        for w in ("Wq", "Wk", "Wv", "Wp")
    }
    for bname in ("bq", "bk", "bv", "bp"):
        # all-zero per the problem spec; declared so the harness can bind them
        nc.dram_tensor(bname, [E], f32, kind="ExternalInput")
    out_d = nc.dram_tensor("out", [BL, N, E], f32, kind="ExternalOutput").ap()

    with tile.TileContext(nc) as tc, ExitStack() as ctx:
        wpool = ctx.enter_context(tc.tile_pool(name="wts", bufs=1))
        dram = ctx.enter_context(tc.tile_pool(name="dram", bufs=1, space="DRAM"))
        actp = ctx.enter_context(tc.tile_pool(name="acts", bufs=1))
        smp = ctx.enter_context(tc.tile_pool(name="softmax", bufs=3))
        outp = ctx.enter_context(tc.tile_pool(name="outs", bufs=3))
        psp = ctx.enter_context(tc.tile_pool(name="ps", bufs=2, space="PSUM"))

        ones64 = wpool.tile([P, 64], bf16)
        nc.gpsimd.memset(ones64[:], 1.0)

        # ---- weights: DRAM cast f32->bf16, then transpose-load wT [ei, eo] --
        wT = {}
        for name in ("Wq", "Wk", "Wv", "Wp"):
            wbf = dram.tile([E, E], bf16, tag=f"wbf_{name}", name=f"wbf_{name}")
            nc.gpsimd.dma_start(wbf[:], w_d[name][:])
            tiles = []
            for et in range(ET):
                t = wpool.tile(
                    [P, E], bf16, tag=f"wT_{name}_{et}", name=f"wT_{name}_{et}"
                )
                nc.sync.dma_start(
                    t[:], wbf[:, et * P : (et + 1) * P], transpose=True
                )
                tiles.append(t)
            wT[name] = tiles

        def cast_load_transpose(x_dram, b, tag, ets):
            """DRAM [N, ets*128] f32 -> DRAM bf16 bounce -> ets transposed
            bf16 SBUF tiles [128, N] via DRAM-side transpose-loads."""
            ecols = ets * P
            xbf = dram.tile([N, ecols], bf16, tag=f"bf_{tag}", name=f"bf_{tag}")
            nc.gpsimd.dma_start(xbf[:], x_dram[b])
            tiles = []
            for et in range(ets):
                t = actp.tile(
                    [P, N], bf16, tag=f"{tag}_{et}", name=f"t_{tag}_{et}"
                )
                nc.sync.dma_start(
                    t[:], xbf[:, et * P : (et + 1) * P], transpose=True
                )
                tiles.append(t)
            return tiles

        for b in range(BL):
            # ---- load + transpose activations ----
            vT = cast_load_transpose(v_d, b, "vT", ET)
            qT = cast_load_transpose(q_d, b, "qT", ET)
            kT = cast_load_transpose(k_d, b, "kT", ET)
            dT = cast_load_transpose(d_d, b, "dT", NT)

            # g = exp(d^T), f = d^T * exp(d^T)
            gT, fT = [], []
            for mt in range(NT):
                g = actp.tile([P, N], bf16, tag=f"gT_{mt}", name=f"gT_{mt}")
                nc.scalar.activation(g[:], dT[mt][:], AF.Exp)
                f = actp.tile([P, N], bf16, tag=f"fT_{mt}", name=f"fT_{mt}")
                nc.vector.tensor_mul(f[:], g[:], dT[mt][:])
                gT.append(g)
                fT.append(f)

            # ---- projections (both 512-chunks share one 2-bank psum) ----
            hT = {}
            for xname, xT, wname in (("q", qT, "Wq"), ("k", kT, "Wk")):
                tiles = []
                for ot in range(ET):
                    tiles.append(
                        actp.tile(
                            [P, N],
                            bf16,
                            tag=f"hT_{xname}_{ot}",
                            name=f"hT_{xname}_{ot}",
                        )
                    )
                for ot in range(ET):
                    ps = psp.tile([P, 1024], f32, tag="ps_pair", bufs=3)
                    for nch in range(NC2):
                        for et in range(ET):
                            nc.tensor.matmul(
                                ps[:, nch * 512 : (nch + 1) * 512],
                                wT[wname][et][:, ot * P : (ot + 1) * P],
                                xT[et][:, nch * 512 : (nch + 1) * 512],
                                start=(et == 0),
                                stop=(et == ET - 1),
                            )
                    if xname == "q":
                        nc.vector.tensor_scalar_mul(
                            tiles[ot][:], ps[:], 1.0 / (DH**0.5)
                        )
                    else:
                        nc.vector.tensor_copy(tiles[ot][:], ps[:])
                hT[xname] = tiles

            # vh as one [128, NT*E] tensor; mt pairs share one 2-bank psum
            vh_all = actp.tile([P, NT * E], bf16, tag="vh_all", name="vh_all")
            for mtp in range(NT // 2):
                ps = psp.tile([P, 1024], f32, tag="ps_pair", bufs=3)
                for j in range(2):
                    mt = 2 * mtp + j
                    for et in range(ET):
                        nc.tensor.matmul(
                            ps[:, j * 512 : (j + 1) * 512],
                            vT[et][:, mt * P : (mt + 1) * P],
                            wT["Wv"][et][:, :],
                            start=(et == 0),
                            stop=(et == ET - 1),
                        )
                nc.vector.tensor_copy(
                    vh_all[:, mtp * 1024 : (mtp + 1) * 1024], ps[:]
                )

            # ---- attention: head pairs in one [128,1024] pipeline ----
            x_all = actp.tile([P, HP * N], bf16, tag="x_all", name="x_all")
            sums_all = actp.tile(
                [P, H * 512], f32, tag="sums_all", name="sums_all"
            )
            xu_all = actp.tile([P, H * 512], bf16, tag="xu_all", name="xu_all")
            for hp in range(HP):
                h0, h1 = 2 * hp, 2 * hp + 1
                for ncc in range(NC2):
                    nsl = slice(ncc * 512, (ncc + 1) * 512)
                    slot = hp * 2 + ncc
                    ps_sum = psp.tile([P, 512], f32, tag="ps_sum", bufs=1)
                    ps_x = psp.tile([P, 512], f32, tag="ps_x", bufs=1)

                    def emit_scores(mt):
                        msl = slice(mt * P, (mt + 1) * P)
                        pp = psp.tile(
                            [P, 1024], f32, tag="ps_pair", bufs=3,
                            name=f"pp_{hp}_{ncc}_{mt}",
                        )
                        nc.tensor.matmul(
                            pp[:, 0:512],
                            hT["k"][hp][0:64, msl],
                            hT["q"][hp][0:64, nsl],
                            start=True, stop=True,
                        )
                        nc.tensor.matmul(
                            pp[:, 512:1024],
                            hT["k"][hp][64:128, msl],
                            hT["q"][hp][64:128, nsl],
                            start=True, stop=True,
                        )
                        return pp

                    pps = [emit_scores(0), emit_scores(1)]
                    for mt in range(NT):
                        pp = pps[mt % 2] if False else pps.pop(0)
                        e01 = smp.tile([P, 1024], bf16, tag="e01")
                        nc.scalar.activation(e01[:], pp[:], AF.Exp)
                        if mt + 2 < NT:
                            pps.append(emit_scores(mt + 2))
                        gb = (
                            gT[mt][:, nsl]
                            .rearrange("p (o f) -> p o f", o=1)
                            .broadcast_to((P, 2, 512))
                        )
                        fb = (
                            fT[mt][:, nsl]
                            .rearrange("p (o f) -> p o f", o=1)
                            .broadcast_to((P, 2, 512))
                        )
                        e2 = e01[:].rearrange("p (o f) -> p o f", o=2)
                        t1 = smp.tile([P, 1024], bf16, tag="t1")
                        nc.vector.tensor_mul(
                            t1[:].rearrange("p (o f) -> p o f", o=2), e2, gb
                        )
                        t2 = smp.tile([P, 1024], bf16, tag="t2")
                        nc.vector.tensor_mul(
                            t2[:].rearrange("p (o f) -> p o f", o=2), e2, fb
                        )
                        nc.tensor.matmul(
                            ps_sum[0:64, :], ones64[:], t1[:, 0:512],
                            start=(mt == 0), stop=(mt == NT - 1),
                            skip_group_check=True,
                        )
                        nc.tensor.matmul(
                            ps_sum[64:128, :], ones64[:], t1[:, 512:1024],
                            start=(mt == 0), stop=(mt == NT - 1),
                            skip_group_check=True, tile_position=(0, 64),
                        )
                        nc.tensor.matmul(
                            ps_x[0:64, :],
                            vh_all[:, mt * 512 + h0 * 64 : mt * 512 + h0 * 64 + 64],
                            t2[:, 0:512],
                            start=(mt == 0), stop=(mt == NT - 1),
                            skip_group_check=True,
                        )
                        nc.tensor.matmul(
                            ps_x[64:128, :],
                            vh_all[:, mt * 512 + h1 * 64 : mt * 512 + h1 * 64 + 64],
                            t2[:, 512:1024],
                            start=(mt == 0), stop=(mt == NT - 1),
                            skip_group_check=True, tile_position=(0, 64),
                        )
                    nc.vector.tensor_copy(
                        sums_all[:, slot * 512 : (slot + 1) * 512], ps_sum[:]
                    )
                    nc.vector.tensor_copy(
                        xu_all[:, slot * 512 : (slot + 1) * 512], ps_x[:]
                    )
            # batched reciprocal: rec = exp(-ln(sums)); then one normalize op
            lnt = smp.tile([P, H * 512], f32, tag="lnt", bufs=1)
            nc.scalar.activation(lnt[:], sums_all[:], AF.Ln)
            nc.scalar.activation(lnt[:], lnt[:], AF.Exp, scale=-1.0)
            nc.vector.tensor_mul(x_all[:], xu_all[:], lnt[:])

            # ---- output projection (nt pairs share one 2-bank psum) ----
            for ntp in range(NT // 2):
                ps = psp.tile([P, 1024], f32, tag="ps_pair", bufs=3)
                for j in range(2):
                    nt = 2 * ntp + j
                    for hp in range(HP):
                        # x_all[:, hp*N + nt*P : ...] rows = heads 2hp,2hp+1
                        nc.tensor.matmul(
                            ps[:, j * 512 : (j + 1) * 512],
                            x_all[:, hp * N + nt * P : hp * N + (nt + 1) * P],
                            wT["Wp"][hp][:, :],
                            start=(hp == 0),
                            stop=(hp == HP - 1),
                        )
                ot_sb = outp.tile([P, 1024], f32, tag="ot_sb", bufs=2)
                nc.vector.tensor_copy(ot_sb[:], ps[:])
                nc.sync.dma_start(
                    out_d[
                        b, ntp * 2 * P : (ntp + 1) * 2 * P, :
                    ].rearrange("(c p) e -> p c e", p=P),
                    ot_sb[:].rearrange("p (c e) -> p c e", c=2),
                )

    nc.compile()
    return nc


def _get_graph():
    if "nc" not in _graph_cache:
        _graph_cache["nc"] = build_graph()
    return _graph_cache["nc"]


def make_in_maps(full):
    in_maps = []
    for c in range(NCORES):
        bsl = slice(c * BL, (c + 1) * BL)
        m = {
            "q": full["q"][bsl],
            "k": full["k"][bsl],
            "v": full["v"][bsl],
            "d": full["d"][bsl],
        }
        for w in ("Wq", "Wk", "Wv", "Wp", "bq", "bk", "bv", "bp"):
            m[w] = full[w]
        in_maps.append(m)
    return in_maps


def kernel(**inputs):
    from concourse.bass_utils import run_bass_kernel_spmd

    nc = _get_graph()
    full = {
        k: np.ascontiguousarray(np.asarray(v, np.float32))
        for k, v in inputs.items()
    }
    res = run_bass_kernel_spmd(
        nc,
        make_in_maps(full),
        core_ids=list(range(NCORES)),
        trace=bool(os.environ.get("ATTN_TRACE")),
    )
    if res.exec_time_ns is not None:
        _graph_cache["exec_time_ns"] = res.exec_time_ns
        _graph_cache["profile_json"] = res.profile_json
        _graph_cache["trace"] = res.instructions_and_trace
    out = np.concatenate([res.results[c]["out"] for c in range(NCORES)], axis=0)
    return out
